# revision 35
# baseline (speedup 1.0000x reference)
"""DescripNet (DGCNN-style EdgeConv x2 + global attention pooling) on 8 trn2 cores.

Sharding: core c -> (cloud b = c//2, half = c%2). Each core processes 2048
rows of one 4096-point cloud; the full cloud is resident per core for
knn + neighbor gathers. Cross-core comms: AllGather of h1 between the two
halves of each cloud, AllReduce for BatchNorm stats (over all 8 cores) and
for the attention-pooling numerator/denominator.

v2 vs v1 (1.9x in the instruction-cost model: 730us vs 1384us):
 - fat matmuls in float32r (1 cyc/row at FD>=256 vs 4 for fp32; HW
   measured 1.6e-4 rel err). Layer-0 knn scores stay full fp32: 3-d
   coords have tiny distance gaps and f32r noise breaks the ranking.
 - knn: 2-scan top-k (per-segment max8 + per-segment max_index), 9-bit
   local-index packing into candidate mantissas, 64-wide tournament,
   slot lookup -> exact global indices without full-row re-scans
 - single fused per-tile pipeline (a/scores/scans/gather/aggregate) so
   tile t+1 scans overlap tile t gathers across the whole layer
 - c-table in bf16: halves gather DMA and enables 2x bf16 max tree
 - BN stats: all five column sums accumulate on PE via rank-1 matmuls
   into one PSUM bank (only the first matmul uses start=True: a start
   clears has_written for the WHOLE bank, so later regions must
   initialize via overwrite-on-unset instead)
 - HW quirks found: multi-offset indirect gathers pair offsets with
   dst slots in mismatched order (only [128,1] offset columns work);
   scalar-engine Lrelu ignores the alpha operand (applies 0.01), so
   BN-apply + leaky is Identity(scale,bias) + DVE max(x, 0.2x);
   gpsimd generic tensor ops fail the V3 ISA engine check.
"""

import numpy as np

import concourse.bacc as bacc
import concourse.bass as bass
import concourse.mybir as mybir
import concourse.tile as tile
from concourse.bass import IndirectOffsetOnAxis
from concourse.bass_utils import run_bass_kernel_spmd

F32 = mybir.dt.float32
F32R = mybir.dt.float32r
BF16 = mybir.dt.bfloat16
U32 = mybir.dt.uint32
AX = mybir.AxisListType
OP = mybir.AluOpType
ACTF = mybir.ActivationFunctionType

B = 4
N = 4096
K = 20
HALF = N // 2          # rows per core
NT = HALF // 128       # 16 row tiles of 128
NSEG = 8
SEG = N // NSEG        # 512
NJC = N // 128         # 32 j-chunks for table builds
EPS = 1e-5
SLOPE = 0.2
NEG = -3.0e38
BNK = float(B * N * K)

D0I, D0 = 3, 64
D1I, D1 = 64, 256
DF, DOUT = 512, 256

N_CORES = 8

# knobs
SCORES_F32R = True      # score matmuls in f32r (4x PE speed, ~1e-4 rel noise)
CT_BF16 = True          # c-table in bf16 (half gather DMA, 2x DVE max tree)
L1_AGG_GPSIMD = False    # layer-1 max-aggregation tree on gpsimd


def _bn_stats_and_affine(nc, tc, pool, dout, suma, sumasq, sumasp, sums, qtot,
                         g_row, be_row, cc_in, cc_out, tag, cc_fn):
    """Combine per-core partial sums into BN scale/shift rows [1, dout]."""
    # sum_m = K*suma + sums ; sum_m2 = K*sumasq + 2*sumasp + qtot
    pack = pool.tile([1, 2 * dout], F32, tag=f"bnpack{tag}")
    nc.vector.scalar_tensor_tensor(pack[:, 0:dout], suma[:, :], float(K), sums[:, :],
                                   op0=OP.mult, op1=OP.add)
    tmp = pool.tile([1, dout], F32, tag=f"bntmp{tag}")
    nc.vector.scalar_tensor_tensor(tmp[:, :], sumasp[:, :], 2.0, qtot[:, :],
                                   op0=OP.mult, op1=OP.add)
    nc.vector.scalar_tensor_tensor(pack[:, dout:2 * dout], sumasq[:, :], float(K),
                                   tmp[:, :], op0=OP.mult, op1=OP.add)
    nc.sync.dma_start(cc_in[:, :], pack[:, :])
    cc_fn("AllReduce", OP.add, [list(range(N_CORES))], cc_in[:, :], cc_out[:, :])
    red = pool.tile([1, 2 * dout], F32, tag=f"bnred{tag}")
    nc.sync.dma_start(red[:, :], cc_out[:, :])

    mu = pool.tile([1, dout], F32, tag=f"bnmu{tag}")
    nc.vector.tensor_scalar_mul(mu[:, :], red[:, 0:dout], 1.0 / BNK)
    e2 = pool.tile([1, dout], F32, tag=f"bne2{tag}")
    nc.vector.tensor_scalar_mul(e2[:, :], red[:, dout:2 * dout], 1.0 / BNK)
    musq = pool.tile([1, dout], F32, tag=f"bnmusq{tag}")
    nc.vector.tensor_mul(musq[:, :], mu[:, :], mu[:, :])
    var = pool.tile([1, dout], F32, tag=f"bnvar{tag}")
    nc.vector.tensor_sub(var[:, :], e2[:, :], musq[:, :])
    nc.vector.tensor_scalar_add(var[:, :], var[:, :], EPS)
    sd = pool.tile([1, dout], F32, tag=f"bnsd{tag}")
    nc.scalar.activation(sd[:, :], var[:, :], ACTF.Sqrt)
    inv = pool.tile([1, dout], F32, tag=f"bninv{tag}")
    nc.vector.reciprocal(inv[:, :], sd[:, :])
    scale = pool.tile([1, dout], F32, tag=f"bnscale{tag}")
    nc.vector.tensor_mul(scale[:, :], g_row[:, :], inv[:, :])
    shift = pool.tile([1, dout], F32, tag=f"bnshift{tag}")
    nc.vector.tensor_mul(shift[:, :], mu[:, :], scale[:, :])
    nc.vector.tensor_sub(shift[:, :], be_row[:, :], shift[:, :])
    return scale, shift


def build_program(debug=False, fake_cc=False):
    nc = bacc.Bacc("TRN2", target_bir_lowering=False, debug=False,
                   num_devices=N_CORES)

    SDT = F32R if SCORES_F32R else F32
    CDT = BF16 if CT_BF16 else F32R

    # ---------------- I/O ----------------
    # feature buffers are consumed by f32r matmuls -> declare DRAM as f32r
    xT_all = nc.dram_tensor("xT_all", [D0I, N], F32R, kind="ExternalInput")
    xT_own = nc.dram_tensor("xT_own", [D0I, HALF], F32R, kind="ExternalInput")
    eye128 = nc.dram_tensor("eye128", [128, 128], F32, kind="ExternalInput")
    eye128r = nc.dram_tensor("eye128r", [128, 128], CDT, kind="ExternalInput")
    ones_col = nc.dram_tensor("ones_col", [128, 1], F32R, kind="ExternalInput")
    ones_row = nc.dram_tensor("ones_row", [1, 128], F32R, kind="ExternalInput")
    ones_n = nc.dram_tensor("ones_n", [1, N], F32R, kind="ExternalInput")
    w_t0 = nc.dram_tensor("w_t0", [D0I + 1, D0], F32R, kind="ExternalInput")
    w_cb0 = nc.dram_tensor("w_cb0", [D0I + 1, D0], F32R, kind="ExternalInput")
    g0_r = nc.dram_tensor("g0_r", [1, D0], F32, kind="ExternalInput")
    be0_r = nc.dram_tensor("be0_r", [1, D0], F32, kind="ExternalInput")
    w_t1 = nc.dram_tensor("w_t1", [D1I + 1, D1], F32R, kind="ExternalInput")
    w_cb1 = nc.dram_tensor("w_cb1", [D1I + 1, D1], F32R, kind="ExternalInput")
    g1_r = nc.dram_tensor("g1_r", [1, D1], F32, kind="ExternalInput")
    be1_r = nc.dram_tensor("be1_r", [1, D1], F32, kind="ExternalInput")
    wg_d = nc.dram_tensor("wg_d", [D1, 1], F32R, kind="ExternalInput")
    bg_d = nc.dram_tensor("bg_d", [1, 1], F32R, kind="ExternalInput")
    wf_d = nc.dram_tensor("wf_d", [D1, DF], F32R, kind="ExternalInput")
    bf_d = nc.dram_tensor("bf_d", [1, DF], F32R, kind="ExternalInput")
    wl_d = nc.dram_tensor("wl_d", [DF, DOUT], F32R, kind="ExternalInput")
    bl_rep = nc.dram_tensor("bl_rep", [B, DOUT], F32, kind="ExternalInput")
    onehot = nc.dram_tensor("onehot", [1, B], F32R, kind="ExternalInput")

    out_d = nc.dram_tensor("out", [B, DOUT], F32, kind="ExternalOutput")
    if debug:
        dbg_idx = nc.dram_tensor("dbg_idx", [128, NT, 24], U32,
                                 kind="ExternalOutput")
        dbg_cand = nc.dram_tensor("dbg_cand", [128, 64], F32,
                                  kind="ExternalOutput")
        dbg_candloc = nc.dram_tensor("dbg_candloc", [128, 64], U32,
                                     kind="ExternalOutput")
        dbg_s = nc.dram_tensor("dbg_s", [128, N], F32, kind="ExternalOutput")
        dbg_z = nc.dram_tensor("dbg_z", [128, NT, D0], F32,
                               kind="ExternalOutput")
        dbg_bn = nc.dram_tensor("dbg_bn", [1, 2 * D0], F32,
                                kind="ExternalOutput")
        dbg_h1 = nc.dram_tensor("dbg_h1", [D1I, HALF], F32,
                                kind="ExternalOutput")
        dbg_sums = nc.dram_tensor("dbg_sums", [1, 5 * D0], F32,
                                  kind="ExternalOutput")
        dbg_g = nc.dram_tensor("dbg_g", [128, K, D0], F32,
                               kind="ExternalOutput")
        dbg_ct = nc.dram_tensor("dbg_ct", [N, D0], F32,
                                kind="ExternalOutput")

    # internal DRAM
    c0_dram = nc.dram_tensor("c0_dram", [N, D0], CDT)
    c1_dram = nc.dram_tensor("c1_dram", [N, D1], CDT)
    h1t_in = nc.dram_tensor("h1t_in", [D1I, HALF], F32R)
    h1t_out = nc.dram_tensor("h1t_out", [2 * D1I, HALF], F32R)
    bn0_in = nc.dram_tensor("bn0_in", [1, 2 * D0], F32)
    bn0_out = nc.dram_tensor("bn0_out", [1, 2 * D0], F32)
    bn1_in = nc.dram_tensor("bn1_in", [1, 2 * D1], F32)
    bn1_out = nc.dram_tensor("bn1_out", [1, 2 * D1], F32)
    pool_in = nc.dram_tensor("pool_in", [B, DF + 1], F32)
    pool_out = nc.dram_tensor("pool_out", [B, DF + 1], F32)

    def _cc(kind, op, groups, cin, cout, nc=nc):
        if not fake_cc:
            nc.gpsimd.collective_compute(
                kind, op, replica_groups=groups,
                ins=[cin.opt()], outs=[cout.opt()])
        else:
            nc.sync.dma_start(cout, cin)

    with tile.TileContext(nc) as tc:
        with tc.tile_pool(name="persist", bufs=1) as P:

            eye = P.tile([128, 128], F32)
            nc.sync.dma_start(eye[:, :], eye128[:, :])
            eye_c = P.tile([128, 128], CDT)
            nc.sync.dma_start(eye_c[:, :], eye128r[:, :])
            onec = P.tile([128, 1], F32R)
            nc.sync.dma_start(onec[:, :], ones_col[:, :])
            oner = P.tile([1, 128], F32R)
            nc.sync.dma_start(oner[:, :], ones_row[:, :])

            idxb = P.tile([128, NT, 24], U32)

            # ======================================================
            def edgeconv_layer(lyr, LP, din, dout, buf_own, buf_a, buf_b,
                               wts, wcb, g_dr, be_dr, c_dram,
                               cc_in, cc_out):
                """One EdgeConv layer. buf_own [din+1, HALF] (row0=1),
                buf_a [din+1, N] (row0=1), buf_b [din+1, N] (row0=-0.5 d2).
                Returns (z_sb [128, NT, dout] f32, scale, shift rows)."""

                # ---- c table -> DRAM (+ sbuf bf16 chunks) ----
                with tc.tile_pool(name=f"ctab{lyr}", bufs=3) as CP, \
                     tc.tile_pool(name=f"ctabp{lyr}", bufs=2, space="PSUM") as CPP:
                    for jc in range(NJC):
                        ps = CPP.tile([128, dout], F32)
                        nc.tensor.matmul(ps[:, :],
                                         buf_a[:, jc * 128:(jc + 1) * 128],
                                         wcb[:, :])
                        ct = CP.tile([128, dout], CDT)
                        nc.scalar.activation(ct[:, :], ps[:, :], ACTF.Copy)
                        nc.sync.dma_start(c_dram[jc * 128:(jc + 1) * 128, :],
                                          ct[:, :])

                # ---- fused per-tile pipeline ----
                z_sb = LP.tile([128, NT, dout], CDT, tag=f"z{lyr}")
                HK = K // 2
                agg_eng = nc.gpsimd if (L1_AGG_GPSIMD and lyr == 1) else nc.vector
                PSTAT_cm = tc.tile_pool(name=f"psStat{lyr}", bufs=1, space="PSUM")
                PST = PSTAT_cm.__enter__()
                pk = PST.tile([1, 5 * dout], F32, tag="pk")
                ps_suma = pk[:, 0:dout]
                ps_sumasq = pk[:, dout:2 * dout]
                ps_sums = pk[:, 2 * dout:3 * dout]
                ps_sumasp = pk[:, 3 * dout:4 * dout]
                ps_q = pk[:, 4 * dout:5 * dout]
                with tc.tile_pool(name=f"atab{lyr}", bufs=1, space="PSUM") as APP, \
                     tc.tile_pool(name=f"asq{lyr}", bufs=2) as AQ, \
                     tc.tile_pool(name=f"ssb{lyr}", bufs=2) as SP, \
                     tc.tile_pool(name=f"cand{lyr}", bufs=2) as CAND, \
                     tc.tile_pool(name=f"selps{lyr}", bufs=2, space="PSUM") as SPP, \
                     tc.tile_pool(name=f"g{lyr}", bufs=2) as GP, \
                     tc.tile_pool(name=f"gsq{lyr}", bufs=1) as GQ, \
                     tc.tile_pool(name=f"ssl{lyr}", bufs=2) as SL, \
                     tc.tile_pool(name=f"aggps{lyr}", bufs=2, space="PSUM") as AGP:
                    for t in range(NT):
                        # a table chunk + suma/sumasq PE accumulation
                        ps_a = APP.tile([128, dout], F32)
                        nc.tensor.matmul(ps_a[:, :],
                                         buf_own[:, t * 128:(t + 1) * 128],
                                         wts[:, :])
                        a_t = AQ.tile([128, dout], F32R, tag="a_t")
                        nc.scalar.activation(a_t[:, :], ps_a[:, :], ACTF.Copy)
                        asq = AQ.tile([128, dout], F32R, tag="asq")
                        nc.scalar.activation(asq[:, :],
                                             a_t[:, :].bitcast(F32),
                                             ACTF.Square)
                        nc.tensor.matmul(ps_suma, onec[:, :], a_t[:, :],
                                         start=(t == 0), stop=(t == NT - 1),
                                         skip_group_check=True)
                        nc.tensor.matmul(ps_sumasq, onec[:, :], asq[:, :],
                                         start=False, stop=(t == NT - 1),
                                         skip_group_check=True)

                        # scores (layer 0 in full fp32: 3-d coords have tiny
                        # distance gaps; f32r noise breaks knn ranking there)
                        s_sb = SP.tile([128, N], F32, tag="s")
                        if lyr == 0:
                            lhs = buf_own[:, t * 128:(t + 1) * 128].bitcast(F32)
                        else:
                            lhs = buf_own[:, t * 128:(t + 1) * 128]
                        for q in range(8):
                            ps = SPP.tile([128, 512], F32, tag="sq")
                            c0 = q * 512
                            rhs = buf_b[:, c0:c0 + 512]
                            if lyr == 0:
                                rhs = rhs.bitcast(F32)
                            nc.tensor.matmul(ps[:, :], lhs, rhs)
                            nc.scalar.activation(s_sb[:, c0:c0 + 512],
                                                 ps[:, :], ACTF.Copy)
                        # scan A: per-segment top-8 values (descending)
                        cand = CAND.tile([128, 64], F32, tag="cand")
                        for sg in range(NSEG):
                            nc.vector.max(cand[:, sg * 8:(sg + 1) * 8],
                                          s_sb[:, sg * SEG:(sg + 1) * SEG])
                        # scan B: per-segment indices of those values
                        candloc = CAND.tile([128, 64], U32, tag="candloc")
                        for sg in range(NSEG):
                            nc.vector.max_index(candloc[:, sg * 8:(sg + 1) * 8],
                                                cand[:, sg * 8:(sg + 1) * 8],
                                                s_sb[:, sg * SEG:(sg + 1) * SEG])
                        if debug and lyr == 0 and t == 0:
                            nc.sync.dma_start(dbg_s[:, :], s_sb[:, :])
                            nc.sync.dma_start(dbg_cand[:, :], cand[:, :])
                            nc.sync.dma_start(dbg_candloc[:, :], candloc[:, :])
                        # pack 9-bit local index into candidate mantissa
                        candm = CAND.tile([128, 64], U32, tag="candm")
                        nc.vector.tensor_scalar(candm[:, :],
                                                cand[:, :].bitcast(U32),
                                                0xFFFFFE00, None,
                                                op0=OP.bitwise_and)
                        candp = CAND.tile([128, 64], F32, tag="candp")
                        nc.vector.tensor_tensor(candp[:, :].bitcast(U32),
                                                candm[:, :], candloc[:, :],
                                                op=OP.bitwise_or)
                        # tournament: top-24 of 64 (3 rounds of max8)
                        w24 = CAND.tile([128, 24], F32, tag="w24")
                        slots = CAND.tile([128, 24], U32, tag="slots")
                        cw = candp
                        for r in range(3):
                            nc.vector.max(w24[:, r * 8:(r + 1) * 8], cw[:, :])
                            nc.vector.max_index(slots[:, r * 8:(r + 1) * 8],
                                                w24[:, r * 8:(r + 1) * 8],
                                                candp[:, :])
                            if r < 2:
                                cn = CAND.tile([128, 64], F32, tag=f"cw{r + 1}")
                                nc.vector.match_replace(cn[:, :],
                                                        w24[:, r * 8:(r + 1) * 8],
                                                        cw[:, :], NEG)
                                cw = cn
                        # global idx = (slot>>3)<<9 | (packed & 0x1FF)
                        tA = CAND.tile([128, 24], U32, tag="tA")
                        nc.vector.tensor_scalar(tA[:, :], slots[:, :], 3, None,
                                                op0=OP.logical_shift_right)
                        tB = CAND.tile([128, 24], U32, tag="tB")
                        nc.vector.tensor_scalar(tB[:, :], tA[:, :], 9, None,
                                                op0=OP.logical_shift_left)
                        tC = CAND.tile([128, 24], U32, tag="tC")
                        nc.vector.tensor_scalar(tC[:, :],
                                                w24[:, :].bitcast(U32),
                                                0x1FF, None,
                                                op0=OP.bitwise_and)
                        nc.vector.tensor_tensor(idxb[:, t, :], tB[:, :], tC[:, :],
                                                op=OP.bitwise_or)

                        # gather neighbors (per-rank: [128,1] offsets are the
                        # only form whose desc/offset orders agree on HW)
                        g_all = GP.tile([128, K, dout], CDT, tag="g")
                        for r in range(K):
                            nc.gpsimd.indirect_dma_start(
                                out=g_all[:, r, :],
                                out_offset=None,
                                in_=c_dram[:, :],
                                in_offset=IndirectOffsetOnAxis(
                                    ap=idxb[:, t, r:r + 1], axis=0),
                            )
                        if debug and lyr == 0 and t == 0:
                            gf = GP.tile([128, K, dout], F32, tag="dbgg")
                            nc.vector.tensor_copy(
                                gf.rearrange("p r c -> p (r c)"),
                                g_all.rearrange("p r c -> p (r c)"))
                            nc.sync.dma_start(dbg_g[:, :, :], gf[:, :, :])
                        # max tree over K=20 (pairwise, mostly 2x bf16)
                        m10 = GP.tile([128, 10, dout], CDT, tag="m10")
                        agg_eng.tensor_tensor(
                            m10.rearrange("p r c -> p (r c)"),
                            g_all[:, 0:10, :].rearrange("p r c -> p (r c)"),
                            g_all[:, 10:20, :].rearrange("p r c -> p (r c)"),
                            op=OP.max)
                        m5 = GP.tile([128, 5, dout], CDT, tag="m5")
                        agg_eng.tensor_tensor(
                            m5.rearrange("p r c -> p (r c)"),
                            m10[:, 0:5, :].rearrange("p r c -> p (r c)"),
                            m10[:, 5:10, :].rearrange("p r c -> p (r c)"),
                            op=OP.max)
                        m2 = GP.tile([128, 2, dout], CDT, tag="m2")
                        agg_eng.tensor_tensor(
                            m2.rearrange("p r c -> p (r c)"),
                            m5[:, 0:2, :].rearrange("p r c -> p (r c)"),
                            m5[:, 2:4, :].rearrange("p r c -> p (r c)"),
                            op=OP.max)
                        m1 = GP.tile([128, dout], CDT, tag="m1")
                        agg_eng.tensor_tensor(m1[:, :], m2[:, 0, :], m2[:, 1, :],
                                              op=OP.max)
                        mx_t = GP.tile([128, dout], CDT, tag="mx_t")
                        agg_eng.tensor_tensor(mx_t[:, :], m1[:, :],
                                              m5[:, 4, :], op=OP.max)
                        # z = a + maxagg for this tile (same-dtype TT ins)
                        mx_f = GP.tile([128, dout], F32, tag="mx_f")
                        nc.vector.tensor_copy(mx_f[:, :], mx_t[:, :])
                        nc.vector.tensor_tensor(z_sb[:, t, :],
                                                a_t[:, :].bitcast(F32),
                                                mx_f[:, :], op=OP.add)

                        # ssl = sum_r g  (PE accumulation via identity)
                        ps_sum = AGP.tile([128, dout], F32, tag="pssum")
                        for r in range(K):
                            nc.tensor.matmul(ps_sum[:, :], eye_c[:, :],
                                             g_all[:, r, :],
                                             start=(r == 0), stop=(r == K - 1),
                                             skip_group_check=True)
                        ssl = SL.tile([128, dout], F32R, tag="ssl")
                        nc.scalar.activation(ssl[:, :], ps_sum[:, :], ACTF.Copy)
                        nc.tensor.matmul(ps_sums, onec[:, :], ssl[:, :],
                                         start=False, stop=(t == NT - 1),
                                         skip_group_check=True)
                        # prod = a * ssl -> sumasp on PE
                        prod_f = SL.tile([128, dout], F32, tag="prodf")
                        nc.vector.tensor_tensor(prod_f[:, :],
                                                a_t[:, :].bitcast(F32),
                                                ssl[:, :].bitcast(F32),
                                                op=OP.mult)
                        prod = SL.tile([128, dout], F32R, tag="prod")
                        nc.scalar.activation(prod[:, :], prod_f[:, :], ACTF.Copy)
                        nc.tensor.matmul(ps_sumasp, onec[:, :], prod[:, :],
                                         start=False, stop=(t == NT - 1),
                                         skip_group_check=True)
                        # qtot: sum of squares of gathered values
                        for h in range(2):
                            gsq = GQ.tile([128, HK * dout], F32R, tag="gsq")
                            nc.scalar.activation(
                                gsq[:, :],
                                g_all[:, h * HK:(h + 1) * HK, :].rearrange(
                                    "p r c -> p (r c)"),
                                ACTF.Square)
                            for r in range(HK):
                                nc.tensor.matmul(
                                    ps_q, onec[:, :],
                                    gsq[:, r * dout:(r + 1) * dout],
                                    start=False,
                                    stop=(t == NT - 1 and h == 1 and r == HK - 1),
                                    skip_group_check=True)

                # ---- BN statistics -> scale/shift ----
                suma = LP.tile([1, dout], F32, tag=f"rsuma{lyr}")
                nc.scalar.activation(suma[:, :], ps_suma, ACTF.Copy)
                sumasq = LP.tile([1, dout], F32, tag=f"rsumasq{lyr}")
                nc.scalar.activation(sumasq[:, :], ps_sumasq, ACTF.Copy)
                sumasp = LP.tile([1, dout], F32, tag=f"rsumasp{lyr}")
                nc.scalar.activation(sumasp[:, :], ps_sumasp, ACTF.Copy)
                sums = LP.tile([1, dout], F32, tag=f"rsums{lyr}")
                nc.scalar.activation(sums[:, :], ps_sums, ACTF.Copy)
                qtot = LP.tile([1, dout], F32, tag=f"rqtot{lyr}")
                nc.scalar.activation(qtot[:, :], ps_q, ACTF.Copy)
                if debug and lyr == 0:
                    nc.sync.dma_start(dbg_sums[:, 0:dout], suma[:, :])
                    nc.sync.dma_start(dbg_sums[:, dout:2 * dout], sumasq[:, :])
                    nc.sync.dma_start(dbg_sums[:, 2 * dout:3 * dout], sums[:, :])
                    nc.sync.dma_start(dbg_sums[:, 3 * dout:4 * dout],
                                      sumasp[:, :])
                    nc.sync.dma_start(dbg_sums[:, 4 * dout:5 * dout], qtot[:, :])
                PSTAT_cm.__exit__(None, None, None)

                g_row = LP.tile([1, dout], F32, tag=f"g{lyr}")
                nc.sync.dma_start(g_row[:, :], g_dr[:, :])
                be_row = LP.tile([1, dout], F32, tag=f"be{lyr}")
                nc.sync.dma_start(be_row[:, :], be_dr[:, :])
                scale, shift = _bn_stats_and_affine(
                    nc, tc, LP, dout, suma, sumasq, sumasp, sums, qtot,
                    g_row, be_row, cc_in, cc_out, lyr, _cc)

                return z_sb, scale, shift

            # ======================================================
            # Layer 0
            h1t_own = P.tile([D1I, HALF], F32R)
            with tc.tile_pool(name="l0", bufs=1) as L0P:
                buf_own0 = L0P.tile([D0I + 1, HALF], F32R)
                nc.sync.dma_start(buf_own0[0:1, :], ones_n[:, 0:HALF])
                nc.sync.dma_start(buf_own0[1:D0I + 1, :], xT_own[:, :])
                buf_a0 = L0P.tile([D0I + 1, N], F32R)
                nc.sync.dma_start(buf_a0[0:1, :], ones_n[:, :])
                nc.sync.dma_start(buf_a0[1:D0I + 1, :], xT_all[:, :])
                buf_b0 = L0P.tile([D0I + 1, N], F32R)
                nc.sync.dma_start(buf_b0[1:D0I + 1, :], xT_all[:, :])
                wt0_sb = L0P.tile([D0I + 1, D0], F32R)
                nc.sync.dma_start(wt0_sb[:, :], w_t0[:, :])
                wcb0_sb = L0P.tile([D0I + 1, D0], F32R)
                nc.sync.dma_start(wcb0_sb[:, :], w_cb0[:, :])

                with tc.tile_pool(name="d2p0", bufs=1) as DP, \
                     tc.tile_pool(name="d2ps0", bufs=2, space="PSUM") as DPP:
                    # square includes the ones row -> uniform -0.5 shift per
                    # column; ranking-neutral
                    sq = DP.tile([D0I + 1, N], F32R)
                    nc.scalar.activation(sq[:, :],
                                         buf_a0[:, :].bitcast(F32),
                                         ACTF.Square)
                    for ch in range(N // 512):
                        ps = DPP.tile([1, 512], F32)
                        nc.tensor.matmul(ps[:, :], onec[0:D0I + 1, :],
                                         sq[:, ch * 512:(ch + 1) * 512])
                        nc.scalar.activation(
                            buf_b0[0:1, ch * 512:(ch + 1) * 512],
                            ps[:, :], ACTF.Copy, scale=-0.5)

                z0, scale0, shift0 = edgeconv_layer(
                    0, L0P, D0I, D0, buf_own0, buf_a0, buf_b0,
                    wt0_sb, wcb0_sb, g0_r, be0_r, c0_dram,
                    bn0_in, bn0_out)

                # BN + leaky in transposed domain -> h1t_own [64, HALF] f32r
                with tc.tile_pool(name="bncol0", bufs=1) as BC, \
                     tc.tile_pool(name="bnps0", bufs=2, space="PSUM") as BPP:
                    ps = BPP.tile([D0, 1], F32, tag="sc")
                    nc.tensor.transpose(ps[:, :], scale0[:, :], eye[0:1, 0:1])
                    scol = BC.tile([D0, 1], F32, tag="scol")
                    nc.scalar.activation(scol[:, :], ps[:, :], ACTF.Copy)
                    ps2 = BPP.tile([D0, 1], F32, tag="sh")
                    nc.tensor.transpose(ps2[:, :], shift0[:, :], eye[0:1, 0:1])
                    hcol = BC.tile([D0, 1], F32, tag="hcol")
                    nc.scalar.activation(hcol[:, :], ps2[:, :], ACTF.Copy)
                    with tc.tile_pool(name="trps0", bufs=2, space="PSUM") as TPP, \
                         tc.tile_pool(name="lk0", bufs=2) as LK:
                        for t in range(NT):
                            pst = TPP.tile([D0, 128], CDT)
                            nc.tensor.transpose(pst[:, :], z0[:, t, :],
                                                eye_c[:, :])
                            y = LK.tile([D0, 128], F32, tag="y")
                            nc.scalar.activation(y[:, :], pst[:, :],
                                                 ACTF.Identity,
                                                 scale=scol[:, 0:1],
                                                 bias=hcol[:, 0:1])
                            hh = LK.tile([D0, 128], F32, tag="h")
                            nc.vector.scalar_tensor_tensor(
                                hh[:, :], y[:, :], SLOPE, y[:, :],
                                op0=OP.mult, op1=OP.max)
                            nc.scalar.activation(
                                h1t_own[:, t * 128:(t + 1) * 128], hh[:, :],
                                ACTF.Copy)

            if debug:
                nc.sync.dma_start(dbg_idx[:, :, :], idxb[:, :, :])
                zf = P.tile([128, NT, D0], F32, tag="dbgzf")
                nc.vector.tensor_copy(zf.rearrange("p t c -> p (t c)"),
                                      z0.rearrange("p t c -> p (t c)"))
                nc.sync.dma_start(dbg_z[:, :, :], zf[:, :, :])
                nc.sync.dma_start(dbg_bn[:, 0:D0], scale0[:, :])
                nc.sync.dma_start(dbg_bn[:, D0:2 * D0], shift0[:, :])
                h1f = P.tile([D1I, HALF], F32, tag="dbgh1f")
                nc.vector.tensor_copy(h1f[:, :], h1t_own[:, :].bitcast(F32))
                nc.sync.dma_start(dbg_h1[:, :], h1f[:, :])
            if debug:
                with tc.tile_pool(name="dbgct", bufs=2) as DC:
                    for jc in range(NJC):
                        cb_t = DC.tile([128, D0], CDT, tag="cb")
                        nc.sync.dma_start(cb_t[:, :],
                                          c0_dram[jc * 128:(jc + 1) * 128, :])
                        cf_t = DC.tile([128, D0], F32, tag="cf")
                        nc.vector.tensor_copy(cf_t[:, :], cb_t[:, :])
                        nc.sync.dma_start(dbg_ct[jc * 128:(jc + 1) * 128, :],
                                          cf_t[:, :])
            # exchange halves within each cloud
            nc.sync.dma_start(h1t_in[:, :], h1t_own[:, :])
            if not fake_cc:
                nc.gpsimd.collective_compute(
                    "AllGather", OP.bypass,
                    replica_groups=[[0, 1], [2, 3], [4, 5], [6, 7]],
                    ins=[h1t_in[:, :].opt()], outs=[h1t_out[:, :].opt()],
                )
            else:
                nc.sync.dma_start(h1t_out[0:D1I, :], h1t_in[:, :])
                nc.sync.dma_start(h1t_out[D1I:2 * D1I, :], h1t_in[:, :])

            # Layer 1
            h2t = P.tile([128, 2, NT, 128], F32R)
            with tc.tile_pool(name="l1", bufs=1) as L1P:
                buf_own1 = L1P.tile([D1I + 1, HALF], F32R)
                nc.sync.dma_start(buf_own1[0:1, :], ones_n[:, 0:HALF])
                nc.sync.dma_start(buf_own1[1:D1I + 1, :], h1t_own[:, :])
                buf_a1 = L1P.tile([D1I + 1, N], F32R)
                nc.sync.dma_start(buf_a1[0:1, :], ones_n[:, :])
                nc.sync.dma_start(buf_a1[1:D1I + 1, 0:HALF], h1t_out[0:D1I, :])
                nc.sync.dma_start(buf_a1[1:D1I + 1, HALF:N],
                                  h1t_out[D1I:2 * D1I, :])
                buf_b1 = L1P.tile([D1I + 1, N], F32R)
                nc.sync.dma_start(buf_b1[1:D1I + 1, 0:HALF], h1t_out[0:D1I, :])
                nc.sync.dma_start(buf_b1[1:D1I + 1, HALF:N],
                                  h1t_out[D1I:2 * D1I, :])
                wt1_sb = L1P.tile([D1I + 1, D1], F32R)
                nc.sync.dma_start(wt1_sb[:, :], w_t1[:, :])
                wcb1_sb = L1P.tile([D1I + 1, D1], F32R)
                nc.sync.dma_start(wcb1_sb[:, :], w_cb1[:, :])

                with tc.tile_pool(name="d2p1", bufs=1) as DP, \
                     tc.tile_pool(name="d2ps1", bufs=2, space="PSUM") as DPP:
                    sq = DP.tile([D1I + 1, N], F32R)
                    nc.scalar.activation(sq[:, :],
                                         buf_a1[:, :].bitcast(F32),
                                         ACTF.Square)
                    for ch in range(N // 512):
                        ps = DPP.tile([1, 512], F32)
                        nc.tensor.matmul(ps[:, :], onec[0:D1I + 1, :],
                                         sq[:, ch * 512:(ch + 1) * 512])
                        nc.scalar.activation(
                            buf_b1[0:1, ch * 512:(ch + 1) * 512],
                            ps[:, :], ACTF.Copy, scale=-0.5)

                z1, scale1, shift1 = edgeconv_layer(
                    1, L1P, D1I, D1, buf_own1, buf_a1, buf_b1,
                    wt1_sb, wcb1_sb, g1_r, be1_r, c1_dram,
                    bn1_in, bn1_out)

                # BN + leaky transposed -> h2T [128, 2, NT, 128] f32r
                with tc.tile_pool(name="bncol1", bufs=1) as BC, \
                     tc.tile_pool(name="bnps1", bufs=2, space="PSUM") as BPP:
                    scol = BC.tile([128, 2], F32, tag="scol")
                    hcol = BC.tile([128, 2], F32, tag="hcol")
                    for cb in range(2):
                        ps = BPP.tile([128, 1], F32, tag="sc")
                        nc.tensor.transpose(ps[:, :],
                                            scale1[:, cb * 128:(cb + 1) * 128],
                                            eye[0:1, 0:1])
                        nc.scalar.activation(scol[:, cb:cb + 1], ps[:, :],
                                             ACTF.Copy)
                        ps2 = BPP.tile([128, 1], F32, tag="sh")
                        nc.tensor.transpose(ps2[:, :],
                                            shift1[:, cb * 128:(cb + 1) * 128],
                                            eye[0:1, 0:1])
                        nc.scalar.activation(hcol[:, cb:cb + 1], ps2[:, :],
                                             ACTF.Copy)
                    with tc.tile_pool(name="trps1", bufs=3, space="PSUM") as TPP, \
                         tc.tile_pool(name="lk1", bufs=2) as LK:
                        for t in range(NT):
                            for cb in range(2):
                                pst = TPP.tile([128, 128], CDT)
                                nc.tensor.transpose(
                                    pst[:, :],
                                    z1[:, t, cb * 128:(cb + 1) * 128],
                                    eye_c[:, :])
                                y = LK.tile([128, 128], F32, tag="y")
                                nc.scalar.activation(y[:, :], pst[:, :],
                                                     ACTF.Identity,
                                                     scale=scol[:, cb:cb + 1],
                                                     bias=hcol[:, cb:cb + 1])
                                hh = LK.tile([128, 128], F32, tag="h")
                                nc.vector.scalar_tensor_tensor(
                                    hh[:, :], y[:, :], SLOPE, y[:, :],
                                    op0=OP.mult, op1=OP.max)
                                nc.scalar.activation(h2t[:, cb, t, :],
                                                     hh[:, :], ACTF.Copy)

            # ---------------- attention pooling ----------------
            with tc.tile_pool(name="poolw", bufs=1) as PW, \
                 tc.tile_pool(name="poolsb", bufs=3) as PSB:
                PLOOP_cm = tc.tile_pool(name="poolloop", bufs=1, space="PSUM")
                PACC = PLOOP_cm.__enter__()
                PFF_cm = tc.tile_pool(name="poolff", bufs=2, space="PSUM")
                PFF = PFF_cm.__enter__()
                wg_sb = PW.tile([128, 2, 1], F32R, tag="wg")
                nc.sync.dma_start(wg_sb[:, 0, :], wg_d[0:128, :])
                nc.sync.dma_start(wg_sb[:, 1, :], wg_d[128:256, :])
                wf_sb = PW.tile([128, 2, DF], F32R, tag="wf")
                nc.sync.dma_start(wf_sb[:, 0, :], wf_d[0:128, :])
                nc.sync.dma_start(wf_sb[:, 1, :], wf_d[128:256, :])
                bg_sb = PW.tile([1, 1], F32R, tag="bg")
                nc.sync.dma_start(bg_sb[:, :], bg_d[:, :])
                bf_sb = PW.tile([1, DF], F32R, tag="bf")
                nc.sync.dma_start(bf_sb[:, :], bf_d[:, :])

                e_sb = PSB.tile([128, NT], F32R, tag="e")
                f_sb_p = PSB.tile([128, DF], F32R, tag="fsb")
                ps_num = PACC.tile([1, DF], F32, tag="num")
                ps_den = PACC.tile([1, 1], F32, tag="den")
                for t in range(NT):
                    ps_f = PFF.tile([128, DF], F32, tag="f")
                    ps_g = PFF.tile([128, 1], F32, tag="gt")
                    for cb in range(2):
                        nc.tensor.matmul(ps_f[:, :], h2t[:, cb, t, :],
                                         wf_sb[:, cb, :],
                                         start=(cb == 0), stop=False)
                        nc.tensor.matmul(ps_g[:, :],
                                         h2t[:, cb, t, :].bitcast(F32),
                                         wg_sb[:, cb, :].bitcast(F32),
                                         start=(cb == 0), stop=False)
                    nc.tensor.matmul(ps_f[:, :], oner[:, :], bf_sb[:, :],
                                     start=False, stop=True)
                    nc.tensor.matmul(ps_g[:, :], oner[:, :].bitcast(F32),
                                     bg_sb[:, :].bitcast(F32),
                                     start=False, stop=True)
                    # f = relu(...) on scalar engine
                    nc.scalar.activation(f_sb_p[:, :], ps_f[:, :], ACTF.Relu)
                    # gate = exp(relu(...))
                    gt = PSB.tile([128, 1], F32, tag="gtsb")
                    nc.scalar.activation(gt[:, :], ps_g[:, :], ACTF.Relu)
                    nc.scalar.activation(e_sb[:, t:t + 1], gt[:, :], ACTF.Exp)
                    nc.tensor.matmul(ps_num[:, :], e_sb[:, t:t + 1], f_sb_p[:, :],
                                     start=(t == 0), stop=(t == NT - 1),
                                     skip_group_check=True)
                    nc.tensor.matmul(ps_den[:, :],
                                     e_sb[:, t:t + 1].bitcast(F32),
                                     onec[:, 0:1].bitcast(F32),
                                     start=(t == 0), stop=(t == NT - 1),
                                     skip_group_check=True)

                numden = PSB.tile([1, DF + 1], F32, tag="numden")
                nc.scalar.activation(numden[:, 0:DF], ps_num[:, :], ACTF.Copy)
                nc.scalar.activation(numden[:, DF:DF + 1], ps_den[:, :], ACTF.Copy)
                PFF_cm.__exit__(None, None, None)
                PLOOP_cm.__exit__(None, None, None)

                oh_sb = PW.tile([1, B], F32R, tag="oh")
                nc.sync.dma_start(oh_sb[:, :], onehot[:, :])
                PTAIL_cm = tc.tile_pool(name="pooltail", bufs=1, space="PSUM")
                PTAIL = PTAIL_cm.__enter__()
                numden_r = PSB.tile([1, DF + 1], F32R, tag="numden_r")
                nc.scalar.activation(numden_r[:, :], numden[:, :], ACTF.Copy)
                ps_pp = PTAIL.tile([B, DF + 1], F32, tag="pp")
                nc.tensor.matmul(ps_pp[:, 0:512], oh_sb[:, :], numden_r[:, 0:512])
                nc.tensor.matmul(ps_pp[:, 512:DF + 1], oh_sb[:, :].bitcast(F32),
                                 numden_r[:, 512:DF + 1].bitcast(F32))
                pp_sb = PSB.tile([B, DF + 1], F32, tag="ppsb")
                nc.scalar.activation(pp_sb[:, 0:DF + 1], ps_pp[:, 0:DF + 1],
                                     ACTF.Copy)
                nc.sync.dma_start(pool_in[:, :], pp_sb[:, :])
                _cc("AllReduce", OP.add, [list(range(N_CORES))],
                    pool_in[:, :], pool_out[:, :])
                pall = PSB.tile([B, DF + 1], F32, tag="pall")
                nc.sync.dma_start(pall[:, :], pool_out[:, :])
                recip = PSB.tile([B, 1], F32, tag="recip")
                nc.vector.reciprocal(recip[:, :], pall[:, DF:DF + 1])
                pooled = PSB.tile([B, DF], F32, tag="pooled")
                nc.vector.tensor_scalar_mul(pooled[:, :], pall[:, 0:DF],
                                            recip[:, 0:1])

                # final linear: out = pooled @ Wl + bl
                wl_sb = PW.tile([128, 4, DOUT], F32R, tag="wl")
                for f in range(4):
                    nc.sync.dma_start(wl_sb[:, f, :],
                                      wl_d[f * 128:(f + 1) * 128, :])
                pooledT = PSB.tile([128, 4, B], F32R, tag="pooledT")
                with tc.tile_pool(name="ptps", bufs=2, space="PSUM") as PTP:
                    for f in range(4):
                        ps = PTP.tile([128, B], F32)
                        nc.tensor.transpose(ps[:, :],
                                            pooled[:, f * 128:(f + 1) * 128],
                                            eye[0:B, 0:B])
                        nc.scalar.activation(pooledT[:, f, :], ps[:, :], ACTF.Copy)
                ps_out = PTAIL.tile([B, DOUT], F32, tag="out")
                for f in range(4):
                    nc.tensor.matmul(ps_out[:, :], pooledT[:, f, :],
                                     wl_sb[:, f, :],
                                     start=(f == 0), stop=(f == 3))
                blr = PW.tile([B, DOUT], F32, tag="blr")
                nc.sync.dma_start(blr[:, :], bl_rep[:, :])
                res = PSB.tile([B, DOUT], F32, tag="res")
                nc.vector.tensor_tensor(res[:, :], ps_out[:, :], blr[:, :],
                                        op=OP.add)
                nc.sync.dma_start(out_d[:, :], res[:, :])
                PTAIL_cm.__exit__(None, None, None)

    nc.compile()
    return nc


_NC_CACHE = None


def _get_program():
    global _NC_CACHE
    if _NC_CACHE is None:
        _NC_CACHE = build_program()
    return _NC_CACHE


def make_in_maps(inputs):
    x = np.asarray(inputs["x"], dtype=np.float32)          # [B, N, 3]
    Wt0 = np.asarray(inputs["Wt0"], np.float32)
    bt0 = np.asarray(inputs["bt0"], np.float32)
    Wp0 = np.asarray(inputs["Wp0"], np.float32)
    bp0 = np.asarray(inputs["bp0"], np.float32)
    g0 = np.asarray(inputs["g0"], np.float32)
    be0 = np.asarray(inputs["be0"], np.float32)
    Wt1 = np.asarray(inputs["Wt1"], np.float32)
    bt1 = np.asarray(inputs["bt1"], np.float32)
    Wp1 = np.asarray(inputs["Wp1"], np.float32)
    bp1 = np.asarray(inputs["bp1"], np.float32)
    g1 = np.asarray(inputs["g1"], np.float32)
    be1 = np.asarray(inputs["be1"], np.float32)
    Wg = np.asarray(inputs["Wg"], np.float32)
    bg = np.asarray(inputs["bg"], np.float32)
    Wf = np.asarray(inputs["Wf"], np.float32)
    bf = np.asarray(inputs["bf"], np.float32)
    Wl = np.asarray(inputs["Wl"], np.float32)
    bl = np.asarray(inputs["bl"], np.float32)

    if CT_BF16:
        import ml_dtypes
        eye_r = np.eye(128, dtype=np.float32).astype(ml_dtypes.bfloat16)
    else:
        eye_r = np.eye(128, dtype=np.float32)
    common = {
        "eye128": np.eye(128, dtype=np.float32),
        "eye128r": eye_r,
        "ones_col": np.ones((128, 1), np.float32),
        "ones_row": np.ones((1, 128), np.float32),
        "ones_n": np.ones((1, N), np.float32),
        "w_t0": np.concatenate([np.zeros((1, D0), np.float32), Wt0], axis=0),
        "w_cb0": np.concatenate([(bt0 + bp0)[None, :], Wp0 - Wt0], axis=0),
        "g0_r": g0[None, :], "be0_r": be0[None, :],
        "w_t1": np.concatenate([np.zeros((1, D1), np.float32), Wt1], axis=0),
        "w_cb1": np.concatenate([(bt1 + bp1)[None, :], Wp1 - Wt1], axis=0),
        "g1_r": g1[None, :], "be1_r": be1[None, :],
        "wg_d": Wg, "bg_d": bg[None, :],
        "wf_d": Wf, "bf_d": bf[None, :],
        "wl_d": Wl, "bl_rep": np.broadcast_to(bl[None, :], (B, DOUT)).copy(),
    }

    in_maps = []
    for c in range(N_CORES):
        b, h = c // 2, c % 2
        xT_all = np.ascontiguousarray(x[b].T)                       # [3, N]
        xT_own = np.ascontiguousarray(x[b, h * HALF:(h + 1) * HALF].T)
        oh = np.zeros((1, B), np.float32)
        oh[0, b] = 1.0
        m = dict(common)
        m["xT_all"] = xT_all
        m["xT_own"] = xT_own
        m["onehot"] = oh
        in_maps.append(m)
    return in_maps


def kernel(**inputs):
    in_maps = make_in_maps(inputs)
    nc = _get_program()
    res = run_bass_kernel_spmd(nc, in_maps, core_ids=list(range(N_CORES)))
    return res.results[0]["out"].astype(np.float32)
